# revision 25
# baseline (speedup 1.0000x reference)
"""Trainium2 Bass kernel for the nn_Aggregate GNN message-passing problem.

Computation (see reference):
    keep = (A > 0) limited to the first `neibor_num` set entries per row
    nb_mean = (keep @ X) / max(cnt, 1)
    out = leaky_relu(X @ W_line.T + b_line)
        + where(cnt > 0, leaky_relu(nb_mean @ W_nb.T + b_nb), 0)

Sharding: rows of A / output rows are split across 8 cores (1024 rows
each); no collectives.  Fast-path structural fact (host-verified, numpy
fallback otherwise): every row reaches `neibor_num` set bits within the
first C=256 columns, so the keep mask is confined to A[:, :C] and
cnt == nn for every row.

The kernel computes the TRANSPOSED output outT[cout, row]:
  * biases become per-partition vectors -> ACT's native activation bias
    (out = Lrelu(in*scale + bias)); no rank-1 bias matmuls for xi.
  * the mask/neighbor pipeline runs fp8 with DoubleRow (2 k-tiles per
    matmul instruction = 2x FLOPs/instr measured on hw).
  * xj's bias and the 1/nn mean scale are folded into the operands:
    wnbt = 16*W_nb, mask values are BETA=2^-6 (the smallest normal
    e4m3), the Xw quantize op scales by GAMMA=1/(16*BETA*nn), so
    psJ = keep@Xw IS the xj pre-activation -- the DVE leaky is a
    single max(psJ, .01*psJ) with no scale pass.

Stages per core (R=1024 rows as 2 groups g of 512; C=256 cands as 2
chunks t; Cin=Cout=512 as 4 k-chunks m / 4 cout-chunks c):
  1. cumT(t,g): DoubleRow prefix-count        (PE fp8, [ones|ltri] trick)
  2. keepT = (cumT <= nn) * atT               (DVE -> fp8)
  3. Xw = X_head @ (16*W_nb.T) + 16*b_nb      (PE fp8 DR + rank-1) -> fp8
  4. psJ(c,g) = Xw.T @ keepT                  (PE, one DR matmul each)
     xjL = max(s, .01*s), s = SJ*psJ          (DVE, fp16)
  5. psI(c,g) = W_line @ X_blk.T              (PE fp16, 4 matmuls)
     ot = Lrelu(psI + b_line_c)               (ACT, per-partition bias)
  6. ot += xjL                                (gpsimd accum-DMA per c;
     c3 via two Pool adds to shorten the tail)
  7. store ot [128,1024] fp16; host transposes + upcasts.

Engine queues (in-order per engine; order matters):
  sync(SP):   dma smq, at, wnbt, xht, then the 4 output stores
  scalar(ACT):dma row1, bls, wlt, xt, then the 8 Lrelus
  vector(DVE):keep(g0)x2, xwq x2, keep(g1)x2, xjL x8
  gpsimd:     3 accum-DMAs + 2 adds for c3
  PE:         prefix g0 | Xw | prefix g1 | xj(g0) | xi(c0) | xj(g1)
              | xi(c1) | xi(c2) | xi(c3)
"""

import numpy as np

NCORES = 8
N = 8192
CIN = 512
COUT = 512
R = N // NCORES          # rows per core
C = 256                  # neighbor-candidate column window
NEG = 0.01               # jax.nn.leaky_relu default slope
ALPHA = 16.0             # wnbt = ALPHA * W_nb (keeps W_nb out of fp8 subnorms)
BETA = 2.0 ** -6         # mask value: the smallest NORMAL e4m3 number
# residual scale GAMMA = 1/(ALPHA*BETA*nn) rides the Xw quantize op, so
# psJ == the xj pre-activation exactly: BETA*GAMMA*ALPHA*nn == 1.

_nc_cache = {}
LAST_RESULT = None       # BassKernelResults of the most recent device run
SIM_SAFE = False         # CoreSim lacks Lrelu; True swaps in Identity+DVE max
XJ_TWO_OP = False        # True: avoid both-PSUM-operand DVE leaky (fallback)
ACCUM_DMA = True         # False: all adds on the Pool engine (fallback)


def _build_nc(nn: int):
    import concourse.bass as bass
    import concourse.bacc as bacc
    import concourse.mybir as mybir
    import concourse.tile as tile

    F32 = mybir.dt.float32
    FP16 = mybir.dt.float16
    FP8 = mybir.dt.float8e4
    AF = mybir.ActivationFunctionType
    OP = mybir.AluOpType
    DR = mybir.MatmulPerfMode.DoubleRow
    gam = 1.0 / (ALPHA * BETA * nn)     # Xw quantize scale

    nc = bacc.Bacc("TRN2", target_bir_lowering=False, debug=False)

    at_d = nc.dram_tensor("at", [128, 2048], FP8, kind="ExternalInput")
    smq_d = nc.dram_tensor("smq", [128, 384], FP8, kind="ExternalInput")
    xht_d = nc.dram_tensor("xht", [128, 1024], FP8, kind="ExternalInput")
    wnbt_d = nc.dram_tensor("wnbt", [128, 2048], FP8, kind="ExternalInput")
    row1_d = nc.dram_tensor("row1", [1, 640], FP16, kind="ExternalInput")
    bls_d = nc.dram_tensor("bls", [128, 4], F32, kind="ExternalInput")
    wlt_d = nc.dram_tensor("wlt", [128, 2048], FP16, kind="ExternalInput")
    xt_d = nc.dram_tensor("xt", [128, 4096], FP16, kind="ExternalInput")
    out_d = nc.dram_tensor("out", [512, 1024], FP16, kind="ExternalOutput")

    with tile.TileContext(nc) as tc:
        with (
            tc.tile_pool(name="const", bufs=1) as constp,
            tc.tile_pool(name="eph", bufs=2) as ephp,
            tc.tile_pool(name="xjp", bufs=1) as xjp,
            tc.tile_pool(name="outp", bufs=2) as outp,
            tc.tile_pool(name="psA", bufs=4, space=bass.MemorySpace.PSUM) as psA,
            tc.tile_pool(name="psB", bufs=4, space=bass.MemorySpace.PSUM) as psB,
        ):
            # --- DMA triggers.  sync ring: only the prefix-gating mask
            # operands (smallest, most latency-critical); g0 chunk first.
            smq = constp.tile([128, 3, 128], FP8, name="smq")
            nc.sync.dma_start(smq[:], smq_d[:])
            at = constp.tile([128, 2, 2, 512], FP8, name="at")
            for g in range(2):
                nc.sync.dma_start(at[:, g], at_d[:, g * 1024:(g + 1) * 1024])
            # scalar ring: Xw operands first, then the xi-path bulk
            xht = constp.tile([128, 4, 256], FP8, name="xht")
            nc.scalar.dma_start(xht[:], xht_d[:])
            wnbt = constp.tile([128, 4, 512], FP8, name="wnbt")
            nc.scalar.dma_start(wnbt[:], wnbt_d[:])
            row1 = constp.tile([1, 640], FP16, name="row1")
            nc.scalar.dma_start(row1[:], row1_d[:])
            bls = constp.tile([128, 4], F32, name="bls")
            nc.scalar.dma_start(bls[:], bls_d[:])
            wlt = constp.tile([128, 4, 512], FP16, name="wlt")
            nc.scalar.dma_start(wlt[:], wlt_d[:])
            xt = constp.tile([128, 4, 1024], FP16, name="xt")
            nc.scalar.dma_start(xt[:], xt_d[:])

            # --- 1+2. prefix count (PE DoubleRow) -> keep mask (DVE fp8)
            # smq slots: 0=ones, 1=ltri(=LTRI.T=triu), 2=zero
            keep = constp.tile([128, 2, 1024], FP8, name="keep")

            def emit_prefix(g):
                for t in range(2):
                    cum = psA.tile([128, 512], F32, name="psa", tag="A")
                    lhs = smq[:, 1:3, :] if t == 0 else smq[:, 0:2, :]
                    nc.tensor.matmul(cum[:], lhs, at[:, g], start=True,
                                     stop=True, perf_mode=DR)
                    nc.vector.scalar_tensor_tensor(
                        keep[:, t, g * 512:(g + 1) * 512], cum[:],
                        float(nn) * BETA, at[:, g, t, :],
                        op0=OP.is_le, op1=OP.mult,
                    )

            emit_prefix(0)
            emit_prefix(1)

            # --- 3. Xw[cand, o] = X_head @ (16*W_nb.T) + 16*b_nb, fp8
            # (pw borrows the psB pool: its 4 slots are idle until xi(0))
            xwq = constp.tile([128, 2, 512], FP8, name="xwq")
            for cc in range(2):
                pw = psB.tile([128, 512], F32, name="psi", tag="B")
                for i in range(2):
                    nc.tensor.matmul(
                        pw[:], xht[:, 2 * i:2 * i + 2, cc * 128:(cc + 1) * 128],
                        wnbt[:, 2 * i:2 * i + 2, :],
                        start=(i == 0), stop=False, perf_mode=DR,
                    )
                nc.tensor.matmul(pw[:], row1[:, 512:640], row1[:, 0:512],
                                 start=False, stop=True)
                nc.vector.tensor_scalar(xwq[:, cc, :], pw[:], gam, None,
                                        op0=OP.mult)

            # --- 4+5. neighbor + self linears per (cout chunk c, row group g)
            xjs = [xjp.tile([128, 1024], FP16, name=f"xj{c}") for c in range(4)]
            ots = [outp.tile([128, 1024], FP16, name=f"ot{c}", bufs=1)
                   for c in range(4)]

            def emit_xj(c, g):
                # psJ is already the xj pre-activation (scales folded into
                # wnbt/keep/xwq); drain+leaky is one ACT Lrelu.
                gs = slice(g * 512, (g + 1) * 512)
                pj = psA.tile([128, 512], F32, name="psa", tag="A")
                nc.tensor.matmul(pj[:], xwq[:, 0:2, c * 128:(c + 1) * 128],
                                 keep[:, 0:2, gs], start=True, stop=True,
                                 perf_mode=DR)
                if SIM_SAFE:
                    s = ephp.tile([128, 512], FP16, name="s")
                    nc.vector.tensor_scalar(s[:], pj[:], 1.0, None,
                                            op0=OP.mult)
                    nc.vector.scalar_tensor_tensor(
                        xjs[c][:, gs], s[:], NEG, s[:], op0=OP.mult, op1=OP.max)
                else:
                    nc.scalar.activation(xjs[c][:, gs], pj[:], AF.Lrelu,
                                         alpha=NEG)

            def emit_xi(c):
                pis = []
                for g in range(2):
                    pi = psB.tile([128, 512], F32, name="psi", tag="B")
                    pis.append(pi)
                for m in range(4):
                    for g in range(2):
                        nc.tensor.matmul(
                            pis[g][:], wlt[:, m, c * 128:(c + 1) * 128],
                            xt[:, m, g * 512:(g + 1) * 512],
                            start=(m == 0), stop=(m == 3),
                        )
                for g in range(2):
                    gs = slice(g * 512, (g + 1) * 512)
                    bias = bls[:, c:c + 1]
                    if SIM_SAFE:
                        yi = ephp.tile([128, 512], FP16, name="yi")
                        nc.scalar.activation(yi[:], pis[g][:], AF.Identity,
                                             bias=bias)
                        nc.vector.scalar_tensor_tensor(
                            ots[c][:, gs], yi[:], NEG, yi[:], op0=OP.mult,
                            op1=OP.max)
                    else:
                        nc.scalar.activation(ots[c][:, gs], pis[g][:], AF.Lrelu,
                                             bias=bias, alpha=NEG)

            def emit_finish(c):
                # add the neighbor half on the DVE and store; stores
                # alternate between the two HW DGE rings.
                of = outp.tile([128, 1024], FP16, name="otf", bufs=2)
                for g in range(2):
                    gs = slice(g * 512, (g + 1) * 512)
                    nc.vector.tensor_tensor(of[:, gs], ots[c][:, gs],
                                            xjs[c][:, gs], op=OP.add)
                eng = nc.sync if c % 2 == 0 else nc.scalar
                eng.dma_start(out_d[c * 128:(c + 1) * 128, :], of[:])

            for c in range(4):
                emit_xj(c, 0)
            emit_xi(0)
            for c in range(4):
                emit_xj(c, 1)
            emit_finish(0)
            emit_xi(1)
            emit_finish(1)
            emit_xi(2)
            emit_finish(2)
            emit_xi(3)
            emit_finish(3)

    nc.compile()
    return nc


def _get_nc(nn: int):
    key = (nn, SIM_SAFE, XJ_TWO_OP, ACCUM_DMA)
    if key not in _nc_cache:
        _nc_cache[key] = _build_nc(nn)
    return _nc_cache[key]


def _numpy_fallback(X, A, W_nb, b_nb, W_line, b_line, nn):
    def leaky(x):
        return np.where(x >= 0, x, NEG * x)

    Ab = A > 0
    keep = Ab & (np.cumsum(Ab.astype(np.int64), axis=1) <= nn)
    cnt = keep.sum(axis=1, keepdims=True).astype(X.dtype)
    nb_sum = keep.astype(X.dtype) @ X
    nb_mean = nb_sum / np.maximum(cnt, 1.0)
    xj = leaky(nb_mean @ W_nb.T + b_nb)
    xi = leaky(X @ W_line.T + b_line)
    return (xi + np.where(cnt > 0, xj, 0.0)).astype(np.float32)


def _pack_m(arr, nm):
    """[nm*128, w] -> [128, nm*w]: chunk m lands at columns [m*w:(m+1)*w]."""
    w = arr.shape[1]
    return np.ascontiguousarray(
        arr.reshape(nm, 128, w).transpose(1, 0, 2).reshape(128, nm * w))


def build_in_maps(X, A, W_nb, b_nb, W_line, b_line, nn):
    """Shard the full inputs into one input map per core."""
    import ml_dtypes
    f8 = ml_dtypes.float8_e4m3

    ones = np.ones((128, 128), np.float32)
    smq = np.concatenate(
        [ones, np.triu(ones), np.zeros((128, 128), np.float32)],
        axis=1).astype(f8)                                      # [128, 384]
    xht = _pack_m(np.ascontiguousarray(X[:C].T).astype(f8), 4)  # [128, 1024]
    wnbt = _pack_m(
        np.ascontiguousarray(W_nb.T * np.float32(ALPHA)).astype(f8),
        4)                                                      # [128, 2048]
    wlt = _pack_m(np.ascontiguousarray(W_line.T).astype(np.float16), 4)
    row1 = np.concatenate(
        [(np.float32(ALPHA) * b_nb).astype(np.float16).reshape(1, COUT),
         np.ones((1, 128), np.float16)], axis=1)                # [1, 640]
    bls = np.ascontiguousarray(
        b_line.astype(np.float32).reshape(4, 128).T)            # [128, 4]

    Ab8 = ((A[:, :C] > 0).astype(np.float32)
           * np.float32(BETA)).astype(f8)                       # [N, 256]
    XT = np.ascontiguousarray(X.T.astype(np.float16))           # [512, N]
    in_maps = []
    for cix in range(NCORES):
        rows = slice(cix * R, (cix + 1) * R)
        blk = Ab8[rows]                                         # [1024, 256]
        at = np.ascontiguousarray(
            blk.reshape(2, 512, 2, 128)                         # [g, r', t, p]
               .transpose(3, 0, 2, 1).reshape(128, 2048))       # [p,(g,t,r')]
        xt = _pack_m(np.ascontiguousarray(XT[:, rows]), 4)      # [128, 4096]
        in_maps.append({
            "at": at, "smq": smq, "xht": xht, "wnbt": wnbt,
            "row1": row1, "bls": bls, "wlt": wlt, "xt": xt,
        })
    return in_maps


def _unshard_out(outs):
    """outs: per-core [512, 1024] fp16 outT -> full [N, 512] f32."""
    full = np.stack([np.asarray(o) for o in outs], axis=0)      # [8, 512, 1024]
    return np.ascontiguousarray(
        full.transpose(0, 2, 1).reshape(N, COUT)).astype(np.float32)


def kernel(**inputs) -> np.ndarray:
    global LAST_RESULT
    X = np.ascontiguousarray(np.asarray(inputs["X"], dtype=np.float32))
    A = np.ascontiguousarray(np.asarray(inputs["A"], dtype=np.int32))
    W_nb = np.asarray(inputs["W_nb"], dtype=np.float32)
    b_nb = np.asarray(inputs["b_nb"], dtype=np.float32)
    W_line = np.asarray(inputs["W_line"], dtype=np.float32)
    b_line = np.asarray(inputs["b_line"], dtype=np.float32)
    nn = int(np.asarray(inputs["neibor_num"]))

    # Fast path requires: every row reaches nn set bits within the first C
    # columns (=> keep-mask confined to [:, :C] and cnt == nn > 0 per row).
    fast = (
        X.shape == (N, CIN) and A.shape == (N, N) and 1 <= nn <= C
        and int(np.count_nonzero(A[:, :C] > 0, axis=1).min()) >= nn
    )
    if not fast:
        return _numpy_fallback(X, A, W_nb, b_nb, W_line, b_line, nn)

    import os

    in_maps = build_in_maps(X, A, W_nb, b_nb, W_line, b_line, nn)
    nc = _get_nc(nn)
    if os.environ.get("BASS_TRACE"):
        from concourse.bass_utils import run_bass_kernel_spmd
        res = run_bass_kernel_spmd(nc, in_maps, core_ids=list(range(NCORES)))
        LAST_RESULT = res
        return _unshard_out([r["out"] for r in res.results])
    outs = _run_cached(nc, nn, in_maps)
    return _unshard_out(outs)


_runner_cache = {}


def _run_cached(nc, nn, in_maps):
    """Execute the compiled program on the 8 cores, caching the jitted
    executable across calls (mirrors bass2jax.run_bass_via_pjrt's
    multi-core path; falls back to it on any setup error)."""
    import jax
    import concourse.mybir as mybir
    from concourse import bass2jax

    if nn not in _runner_cache:
        try:
            bass2jax.install_neuronx_cc_hook()
            part_name = (nc.partition_id_tensor.name
                         if nc.partition_id_tensor else None)
            in_names, out_names, out_avals, zero_shapes = [], [], [], []
            for alloc in nc.m.functions[0].allocations:
                if not isinstance(alloc, mybir.MemoryLocationSet):
                    continue
                name = alloc.memorylocations[0].name
                if alloc.kind == "ExternalInput":
                    if name != part_name:
                        in_names.append(name)
                elif alloc.kind == "ExternalOutput":
                    out_names.append(name)
                    np_dt = mybir.dt.np(alloc.dtype)
                    out_avals.append(jax.core.ShapedArray(
                        tuple(alloc.tensor_shape), np_dt))
                    zero_shapes.append((tuple(alloc.tensor_shape), np_dt))
            n_params = len(in_names)
            all_names = tuple(in_names + out_names
                              + ([part_name] if part_name else []))

            def _body(*args):
                operands = list(args)
                if part_name:
                    operands.append(bass2jax.partition_id_tensor())
                outs = bass2jax._bass_exec_p.bind(
                    *operands,
                    out_avals=tuple(out_avals),
                    in_names=all_names,
                    out_names=tuple(out_names),
                    lowering_input_output_aliases=(),
                    sim_require_finite=True,
                    sim_require_nnan=True,
                    nc=nc,
                )
                return tuple(outs)

            from jax.sharding import Mesh, PartitionSpec
            try:
                from jax.experimental.shard_map import shard_map
            except ImportError:
                from jax.shard_map import shard_map
            devices = jax.devices()[:NCORES]
            assert len(devices) == NCORES
            mesh = Mesh(np.asarray(devices), ("core",))
            n_outs = len(out_names)
            sharded = jax.jit(
                shard_map(_body, mesh=mesh,
                          in_specs=(PartitionSpec("core"),) * (n_params + n_outs),
                          out_specs=(PartitionSpec("core"),) * n_outs,
                          check_rep=False),
                donate_argnums=tuple(range(n_params, n_params + n_outs)),
                keep_unused=True,
            )
            _runner_cache[nn] = (sharded, in_names, out_names, zero_shapes)
        except Exception:
            _runner_cache[nn] = None
    cached = _runner_cache[nn]
    if cached is None:
        from concourse.bass_utils import run_bass_kernel_spmd
        res = run_bass_kernel_spmd(nc, in_maps, core_ids=list(range(NCORES)))
        return [r["out"] for r in res.results]
    sharded, in_names, out_names, zero_shapes = cached
    concat_in = [np.concatenate([np.asarray(m[name]) for m in in_maps], axis=0)
                 for name in in_names]
    concat_zeros = [np.zeros((NCORES * sh[0],) + sh[1:], dt)
                    for sh, dt in zero_shapes]
    out_arrs = sharded(*concat_in, *concat_zeros)
    oi = out_names.index("out")
    full = np.asarray(out_arrs[oi]).reshape(NCORES, 512, R)
    return [full[c] for c in range(NCORES)]


if __name__ == "__main__":
    rng = np.random.default_rng(0)
    X = rng.standard_normal((N, CIN), dtype=np.float32)
    A = (rng.random((N, N)) < 0.5).astype(np.int32)
    W_nb = rng.standard_normal((COUT, CIN), dtype=np.float32) * 0.04
    b_nb = rng.standard_normal(COUT, dtype=np.float32) * 0.04
    W_line = rng.standard_normal((COUT, CIN), dtype=np.float32) * 0.04
    b_line = rng.standard_normal(COUT, dtype=np.float32) * 0.04
    out = kernel(X=X, A=A, W_nb=W_nb, b_nb=b_nb, W_line=W_line,
                 b_line=b_line, neibor_num=64)
    exp = _numpy_fallback(X, A, W_nb, b_nb, W_line, b_line, 64)
    err = np.abs(out - exp).max() / np.abs(exp).max()
    print("self-test rel err:", err)
